# revision 31
# baseline (speedup 1.0000x reference)
"""Causal multi-head attention on 8 trn2 NeuronCores.

Sharding: head-parallel. Each core owns 2 of the 16 heads (128 of the 1024
channels) for all 4 batches. Per core:
  Q^T/K^T/V^T projections (local 128 channels) from x^T (host-transposed,
  a pure layout prep like the weight slicing); flash-style causal attention
  in score-transposed layout S^T[k, q]; softmax denominators ride along as a
  ones column appended to V (PV matmul M=65/66, den lands on its own PSUM
  partition); normalization is applied to A^T via a rank-2 "R" matmul built
  from the reciprocals; local Wo row-block matmul produces a full
  [8192, 1024] partial per core, summed (+bias) on host.

Engine-queue discipline (queues are static FIFO on TRN2):
  - The program is software-pipelined at EMISSION level: projections of
    batch b+1 and the Wo of batch b-1 are emitted between the attention
    q-tiles of batch b, so the PE queue interleaves them into the
    ACT-paced attention stretches.
  - The softmax reciprocal (DVE iterative-divide, cost ~ 8 cycles per FREE
    element, partition-parallel) is computed on a [128, 8] repartitioned
    copy of the denominators (tiny DMAs out/back), so it costs ~0.1us of
    DVE queue time instead of 3.3us.

All heavy matmuls run in float32r (tf32-like, full PE rate at free dim >=
256, ~1.5e-4 scale-relative per matmul measured on HW). PE transposes (V^T
-> V) run in plain fp32 (exact). Softmax skips the max-subtraction (scores
are bounded; fp32 exp cannot overflow) and folds the 1/sqrt(64) scale into
the ACT exp. Causal masking is block-wise: off-diagonal key blocks are
skipped entirely; diagonal blocks get a -3e38 triangular mask before exp,
and straddle blocks only compute/exp their valid columns.
"""
import sys

sys.path.insert(0, "/opt/trn_rl_repo")

import numpy as np

import concourse.bass as bass
import concourse.tile as tile
from concourse import bacc, mybir
from concourse.bass_utils import run_bass_kernel_spmd

f32 = mybir.dt.float32
f32r = mybir.dt.float32r
EXP = mybir.ActivationFunctionType.Exp

B, S, D, H, HD = 4, 2048, 1024, 16, 64
NCORES = 8
CLOC = D // NCORES       # 128 local channels = 2 heads per core
BS = B * S               # 8192
QT = 4                   # q tiles of 512 per batch
KB = 16                  # k blocks of 128 per batch
NEG = -3.0e38


def build_program():
    """Build + compile the per-core Bacc program (identical on all cores)."""
    nc = bacc.Bacc("TRN2", target_bir_lowering=False, debug=False)

    xtr_d = nc.dram_tensor("xtr", [D, BS], f32r, kind="ExternalInput").ap()
    wq_d = nc.dram_tensor("wq", [D, CLOC], f32, kind="ExternalInput").ap()
    wk_d = nc.dram_tensor("wk", [D, CLOC], f32, kind="ExternalInput").ap()
    wv_d = nc.dram_tensor("wv", [D, CLOC], f32, kind="ExternalInput").ap()
    wo_d = nc.dram_tensor("wo", [CLOC, D], f32, kind="ExternalInput").ap()
    selc_d = nc.dram_tensor("selc", [2, CLOC], f32, kind="ExternalInput").ap()
    out_d = nc.dram_tensor("out", [BS, D], f32, kind="ExternalOutput").ap()

    with tile.TileContext(nc) as tc:
        _Builder(nc, tc, xtr_d, wq_d, wk_d, wv_d, wo_d, selc_d, out_d).build()
    nc.compile()
    return nc


class _Builder:
    def __init__(self, nc, tc, xtr_d, wq_d, wk_d, wv_d, wo_d, selc_d, out_d):
        self.nc = nc
        self.tc = tc
        self.xtr_d = xtr_d
        self.w_d = {"q": wq_d, "k": wk_d, "v": wv_d}
        self.wo_d = wo_d
        self.selc_d = selc_d
        self.out_d = out_d
        self.st_b = {}   # per-batch state: xt, qT, kT, vT, aT, v_tiles
        from collections import deque
        self.fillers = deque()

    def build(self):
        from contextlib import ExitStack

        nc, tc = self.nc, self.tc
        with ExitStack() as ctx:
            p = self.p = {}
            for name, bufs, space in (
                ("consts", 1, None), ("wpool", 1, None), ("xtp", 1, None),
                ("qkv", 2, None), ("vtpool", 1, None), ("vpp", 24, None),
                ("ptp", 4, None), ("atp", 2, None), ("denp", 3, None),
                ("outp", 2, None),
                ("ps_a", 2, "PSUM"), ("ps_st", 2, "PSUM"),
                ("ps_pv", 2, "PSUM"),
            ):
                kw = {"space": space} if space else {}
                p[name] = ctx.enter_context(
                    tc.tile_pool(name=name, bufs=bufs, **kw))

            self._consts()
            self._weights()

            # ---- software pipeline across batches: proj(b+1)/Wo(b)
            # queue as PE "filler" thunks drained inside the attention
            # kb loop so the PE queue never idles on exp waits ----
            self._xt_dma(0)
            for qt in range(QT):
                self._proj_group(0, qt)
            self._vtrans(0)
            for b in range(B):
                if b + 1 < B:
                    self._xt_dma(b + 1)
                    for qt in range(QT):
                        self._enqueue_proj(b + 1, qt)
                for qt in range(QT):
                    self._attention_qtile(b, qt)
                    self._den_part1(b, qt)
                    if qt >= 1:
                        self._den_part2(b, qt - 1)
                        self._enqueue_wo(b, qt - 1)
                self._den_part2(b, QT - 1)
                self._enqueue_wo(b, QT - 1)
                self._drain_fillers()
            self._drain_fillers()

    # ------------------------------------------------------------------
    def _consts(self):
        nc, p = self.nc, self.p
        ident = p["consts"].tile([128, 128], f32)
        nc.gpsimd.memset(ident[:], 0.0)
        nc.gpsimd.affine_select(
            out=ident[:], in_=ident[:],
            compare_op=mybir.AluOpType.not_equal, fill=1.0, base=0,
            pattern=[[-1, 128]], channel_multiplier=1,
        )
        trimask = p["consts"].tile([128, 128], f32)
        nc.gpsimd.memset(trimask[:], 0.0)
        nc.gpsimd.affine_select(
            out=trimask[:], in_=trimask[:],
            compare_op=mybir.AluOpType.is_ge, fill=NEG, base=0,
            pattern=[[1, 128]], channel_multiplier=-1,
        )
        sel_stg = p["consts"].tile([66, 128], f32)
        nc.sync.dma_start(sel_stg[64:66, :], self.selc_d)
        sel = p["consts"].tile([66, 128], f32r)
        nc.vector.tensor_copy(sel[64:66, :], sel_stg[64:66, :])
        ones_c = p["consts"].tile([128, 1], f32)
        nc.vector.memset(ones_c[:], 1.0)
        self.ident, self.trimask, self.sel, self.ones_c = \
            ident, trimask, sel, ones_c

    def _weights(self):
        nc, p = self.nc, self.p
        self.w_sb = {}
        for name in ("q", "k", "v"):
            stg = p["wpool"].tile([128, D], f32, tag="wstg")
            nc.sync.dma_start(
                stg[:].rearrange("p (c m) -> p c m", c=8),
                self.w_d[name].rearrange("(c p) m -> p c m", p=128))
            w_sb = p["wpool"].tile([128, D], f32r, tag="w_" + name)
            nc.vector.tensor_copy(w_sb[:], stg[:])
            self.w_sb[name] = w_sb
        wo_stg = p["wpool"].tile([128, D], f32, tag="wstg")
        nc.sync.dma_start(wo_stg[:], self.wo_d)
        self.wo_sb = p["wpool"].tile([128, D], f32r, tag="w_o")
        nc.vector.tensor_copy(self.wo_sb[:], wo_stg[:])

    def _st(self, b):
        return self.st_b.setdefault(b, {})

    def _xt_dma(self, b):
        nc, p = self.nc, self.p
        xt = p["xtp"].tile([128, 8 * S], f32r, tag="xt")
        self._st(b)["xt"] = xt
        # qt-major sub-DMAs so the first projection group of this batch
        # only waits for its own 8 slices
        for qt in range(QT):
            for dc in range(8):
                nc.sync.dma_start(
                    xt[:, dc * S + qt * 512: dc * S + (qt + 1) * 512],
                    self.xtr_d[dc * 128:(dc + 1) * 128,
                               b * S + qt * 512: b * S + (qt + 1) * 512])

    def _drain_fillers(self, n=None):
        while self.fillers and (n is None or n > 0):
            self.fillers.popleft()()
            if n is not None:
                n -= 1

    def _enqueue_proj(self, b, qt):
        nc, p = self.nc, self.p
        st = self._st(b)
        if "qT" not in st:
            st["qT"] = p["qkv"].tile([128, S], f32r, tag="qT", name="qT")
            st["kT"] = p["qkv"].tile([128, S], f32r, tag="kT", name="kT")
            st["vT"] = p["vtpool"].tile([128, S], f32, tag="vT", name="vT")
        xt = st["xt"]
        for name in ("v", "q", "k"):
            dst = st[{"q": "qT", "k": "kT", "v": "vT"}[name]]
            if name == "v" and "v_tiles" not in st:
                st["v_tiles"] = [None] * KB
            box = {}

            def mk_mm(dc, name=name, box=box, qt=qt, xt=xt):
                def thunk():
                    if dc == 0:
                        box["pps"] = p["ps_a"].tile(
                            [128, 512], f32, tag="ps_a", name="pps")
                    nc.tensor.matmul(
                        box["pps"][:],
                        self.w_sb[name][:, dc * 128:(dc + 1) * 128],
                        xt[:, dc * S + qt * 512: dc * S + (qt + 1) * 512],
                        start=(dc == 0), stop=(dc == 7))
                return thunk

            for dc in range(8):
                self.fillers.append(mk_mm(dc))

            par = (qt + {"v": 0, "q": 1, "k": 2}[name]) % 2

            def cp(dst=dst, box=box, qt=qt, par=par):
                if par == 0:
                    nc.vector.tensor_copy(
                        dst[:, qt * 512:(qt + 1) * 512], box["pps"][:])
                else:
                    nc.scalar.copy(
                        dst[:, qt * 512:(qt + 1) * 512], box["pps"][:])

            self.fillers.append(cp)
            if name == "v":
                for kb in range(4 * qt, 4 * qt + 4):
                    self.fillers.append(
                        lambda kb=kb, b=b: self._vtrans_one(b, kb))

    def _enqueue_wo(self, b, qt):
        nc, p = self.nc, self.p
        aT = self._st(b)["aT"]
        for qb in range(4 * qt, 4 * qt + 4):
            def thunk(qb=qb, aT=aT, b=b):
                o_sb = p["outp"].tile([128, 1024], f32, tag="osb",
                                      name="osb")
                for nt in range(2):
                    pout = p["ps_a"].tile([128, 512], f32, tag="ps_a",
                                          name="pout")
                    nc.tensor.matmul(
                        pout[:], aT[:, qb * 128:(qb + 1) * 128],
                        self.wo_sb[:, nt * 512:(nt + 1) * 512],
                        start=True, stop=True)
                    if (qb + nt) % 2 == 0:
                        nc.vector.tensor_copy(
                            o_sb[:, nt * 512:(nt + 1) * 512], pout[:])
                    else:
                        nc.scalar.copy(
                            o_sb[:, nt * 512:(nt + 1) * 512], pout[:])
                nc.sync.dma_start(
                    self.out_d[b * S + qb * 128: b * S + (qb + 1) * 128, :],
                    o_sb[:])
            self.fillers.append(thunk)

    def _proj_group(self, b, qt):
        nc, p = self.nc, self.p
        st = self._st(b)
        if "qT" not in st:
            st["qT"] = p["qkv"].tile([128, S], f32r, tag="qT", name="qT")
            st["kT"] = p["qkv"].tile([128, S], f32r, tag="kT", name="kT")
            st["vT"] = p["vtpool"].tile([128, S], f32, tag="vT", name="vT")
        xt = st["xt"]
        for name, dst in (("q", st["qT"]), ("k", st["kT"]), ("v", st["vT"])):
            pps = p["ps_a"].tile([128, 512], f32, tag="ps_a")
            for dc in range(8):
                nc.tensor.matmul(
                    pps[:], self.w_sb[name][:, dc * 128:(dc + 1) * 128],
                    xt[:, dc * S + qt * 512: dc * S + (qt + 1) * 512],
                    start=(dc == 0), stop=(dc == 7))
            nc.vector.tensor_copy(dst[:, qt * 512:(qt + 1) * 512], pps[:])

    def _vtrans(self, b):
        st = self._st(b)
        st.setdefault("v_tiles", [None] * KB)
        for kb in range(KB):
            self._vtrans_one(b, kb)

    def _vtrans_one(self, b, kb):
        nc, p = self.nc, self.p
        st = self._st(b)
        vT = st["vT"]
        tp2 = p["ps_a"].tile([128, 512], f32, tag="ps_a")
        nc.tensor.transpose(
            tp2[:, 0:128], vT[:, kb * 128:(kb + 1) * 128], self.ident[:])
        vt = p["vpp"].tile([128, 131], f32r, tag="vp")
        # [V_A(0:64) | 1(64) | V_B(65:129) | pad(129, unread) | 1(130)]
        nc.vector.tensor_copy(vt[:, 64:65], self.ones_c[:])
        nc.vector.tensor_copy(vt[:, 130:131], self.ones_c[:])
        nc.vector.tensor_copy(vt[:, 0:64], tp2[:, 0:64])
        nc.vector.tensor_copy(vt[:, 65:129], tp2[:, 64:128])
        st["v_tiles"][kb] = vt

    def _attention_qtile(self, b, qt):
        nc, p = self.nc, self.p
        st = self._st(b)
        qT, kT, v_tiles = st["qT"], st["kT"], st["v_tiles"]
        if "aT" not in st:
            st["aT"] = p["atp"].tile([128, S], f32r, tag="aT", name="aT")
        pvA = p["ps_pv"].tile([128, 512], f32, tag="ps_pv")
        pvB = p["ps_pv"].tile([128, 512], f32, tag="ps_pv")
        st["pv"] = (pvA, pvB)
        nkb = 4 * qt + 4
        for kb in range(nkb):
            off = max(0, (kb - 4 * qt) * 128)
            stp = p["ps_st"].tile([128, 1024], f32, tag="ps_st")
            nc.tensor.matmul(
                stp[:, off:512], kT[0:64, kb * 128:(kb + 1) * 128],
                qT[0:64, qt * 512 + off:(qt + 1) * 512],
                start=True, stop=True)
            nc.tensor.matmul(
                stp[:, 512 + off:1024],
                kT[64:128, kb * 128:(kb + 1) * 128],
                qT[64:128, qt * 512 + off:(qt + 1) * 512],
                start=True, stop=True)
            if kb - 4 * qt >= 0:
                for hoff in (0, 512):
                    nc.vector.tensor_add(
                        stp[:, hoff + off:hoff + off + 128],
                        stp[:, hoff + off:hoff + off + 128],
                        self.trimask[:])
            pt = p["ptp"].tile([128, 1024], f32r, tag="pt")
            st_v = stp[:].rearrange("p (h q) -> p h q", h=2)[:, :, off:512]
            pt_v = pt[:].rearrange("p (h q) -> p h q", h=2)[:, :, off:512]
            nc.scalar.activation(pt_v, st_v, EXP, scale=0.125)
            nc.tensor.matmul(
                pvA[0:65, off:512], v_tiles[kb][:, 0:65], pt[:, off:512],
                start=(kb == 0), stop=(kb == nkb - 1))
            nc.tensor.matmul(
                pvB[0:66, off:512], v_tiles[kb][:, 65:131],
                pt[:, 512 + off:1024],
                start=(kb == 0), stop=(kb == nkb - 1))
            self._drain_fillers(4 if kb % 2 == 0 else 3)

    def _den_part1(self, b, qt):
        nc, p = self.nc, self.p
        st = self._st(b)
        pvA, pvB = st.pop("pv")
        # stage psum out^T -> SBUF and den rows; kick the repartition DMAs
        stgA = p["denp"].tile([128, 512], f32r, tag="stgA")
        nc.scalar.copy(stgA[0:64, :], pvA[0:64, :])
        stgB = p["denp"].tile([128, 512], f32r, tag="stgB")
        nc.scalar.copy(stgB[0:64, :], pvB[0:64, :])
        dens = p["denp"].tile([128, 512], f32, tag="dens")
        nc.vector.tensor_copy(dens[64:66, :], pvB[64:66, :])
        nc.vector.tensor_copy(dens[64:65, :], pvA[64:65, :])
        densP = p["denp"].tile([128, 8], f32, tag="densP")
        for h in range(2):
            for qh in range(4):
                nc.sync.dma_start(
                    densP[:, 4 * h + qh: 4 * h + qh + 1],
                    dens[64 + h: 65 + h, 128 * qh: 128 * (qh + 1)])
        st.setdefault("den_pend", {})[qt] = (stgA, stgB, dens, densP)

    def _den_part2(self, b, qt):
        nc, p = self.nc, self.p
        st = self._st(b)
        aT = st["aT"]
        stgA, stgB, dens, densP = st["den_pend"].pop(qt)
        cols = slice(qt * 512, (qt + 1) * 512)
        nc.vector.reciprocal(densP[:], densP[:])
        for h in range(2):
            for qh in range(4):
                nc.sync.dma_start(
                    dens[64 + h: 65 + h, 128 * qh: 128 * (qh + 1)],
                    densP[:, 4 * h + qh: 4 * h + qh + 1])
        recip_r = p["denp"].tile([128, 512], f32r, tag="recip_r")
        nc.scalar.copy(recip_r[64:66, :], dens[64:66, :])
        r_ps = p["ps_a"].tile([128, 512], f32, tag="ps_a")
        nc.tensor.matmul(r_ps[:], self.sel[64:66, :], recip_r[64:66, :],
                         start=True, stop=True)
        nc.sync.dma_start(aT[64:128, cols], stgB[0:64, :])
        nc.vector.tensor_mul(aT[0:64, cols], stgA[0:64, :], r_ps[0:64, :])
        nc.vector.tensor_mul(aT[64:128, cols], aT[64:128, cols],
                             r_ps[64:128, :])

    def _wo_group(self, b, qt):
        nc, p = self.nc, self.p
        aT = self._st(b)["aT"]
        for qb in range(4 * qt, 4 * qt + 4):
            o_sb = p["outp"].tile([128, 1024], f32, tag="osb")
            for nt in range(2):
                pout = p["ps_a"].tile([128, 512], f32, tag="ps_a")
                nc.tensor.matmul(
                    pout[:], aT[:, qb * 128:(qb + 1) * 128],
                    self.wo_sb[:, nt * 512:(nt + 1) * 512],
                    start=True, stop=True)
                if (qb + nt) % 2 == 0:
                    nc.vector.tensor_copy(
                        o_sb[:, nt * 512:(nt + 1) * 512], pout[:])
                else:
                    nc.scalar.copy(
                        o_sb[:, nt * 512:(nt + 1) * 512], pout[:])
            nc.sync.dma_start(
                self.out_d[b * S + qb * 128: b * S + (qb + 1) * 128, :],
                o_sb[:])


_PROGRAM_CACHE = {}


def _get_program():
    if "nc" not in _PROGRAM_CACHE:
        _PROGRAM_CACHE["nc"] = build_program()
    return _PROGRAM_CACHE["nc"]


def make_in_maps(x, Wq, Wk, Wv, Wo):
    x_flat = np.asarray(x, dtype=np.float32).reshape(BS, D)
    xtr = np.ascontiguousarray(x_flat.T)
    sel_const = np.zeros((2, CLOC), dtype=np.float32)
    sel_const[0, 0:64] = 1.0
    sel_const[1, 64:128] = 1.0
    maps = []
    for c in range(NCORES):
        sl = slice(c * CLOC, (c + 1) * CLOC)
        maps.append({
            "xtr": xtr,
            "wq": np.ascontiguousarray(Wq[:, sl], dtype=np.float32),
            "wk": np.ascontiguousarray(Wk[:, sl], dtype=np.float32),
            "wv": np.ascontiguousarray(Wv[:, sl], dtype=np.float32),
            "wo": np.ascontiguousarray(Wo[sl, :], dtype=np.float32),
            "selc": sel_const,
        })
    return maps


def run(x, Wq, Wk, Wv, Wo, bo, trace=False, **kw):
    nc = _get_program()
    maps = make_in_maps(x, Wq, Wk, Wv, Wo)
    res = run_bass_kernel_spmd(nc, maps, core_ids=list(range(NCORES)),
                               trace=trace, **kw)
    acc = res.results[0]["out"].astype(np.float32)
    for c in range(1, NCORES):
        acc = acc + res.results[c]["out"]
    out = (acc + np.asarray(bo, dtype=np.float32)).reshape(B, S, D)
    return out, res


def kernel(x, Wq, Wk, Wv, Wo, bo):
    out, _ = run(x, Wq, Wk, Wv, Wo, bo, trace=False)
    return out


# revision 32
# speedup vs baseline: 1.1994x; 1.1994x over previous
"""Causal multi-head attention on 8 trn2 NeuronCores.

Sharding: head-parallel. Each core owns 2 of the 16 heads (128 of the 1024
channels) for all 4 batches. Per core:
  Q^T/K^T/V^T projections (local 128 channels) from x^T (host-transposed,
  a pure layout prep like the weight slicing); flash-style causal attention
  in score-transposed layout S^T[k, q]; softmax denominators ride along as a
  ones column appended to V (PV matmul M=65/66, den lands on its own PSUM
  partition); normalization is applied to A^T via a rank-2 "R" matmul built
  from the reciprocals; local Wo row-block matmul produces a full
  [8192, 1024] partial per core, summed (+bias) on host.

Engine-queue discipline (queues are static FIFO on TRN2):
  - The program is software-pipelined at EMISSION level: projections of
    batch b+1 and the Wo of batch b-1 are emitted between the attention
    q-tiles of batch b, so the PE queue interleaves them into the
    ACT-paced attention stretches.
  - The softmax reciprocal (DVE iterative-divide, cost ~ 8 cycles per FREE
    element, partition-parallel) is computed on a [128, 8] repartitioned
    copy of the denominators (tiny DMAs out/back), so it costs ~0.1us of
    DVE queue time instead of 3.3us.

All heavy matmuls run in float32r (tf32-like, full PE rate at free dim >=
256, ~1.5e-4 scale-relative per matmul measured on HW). PE transposes (V^T
-> V) run in plain fp32 (exact). Softmax skips the max-subtraction (scores
are bounded; fp32 exp cannot overflow) and folds the 1/sqrt(64) scale into
the ACT exp. Causal masking is block-wise: off-diagonal key blocks are
skipped entirely; diagonal blocks get a -3e38 triangular mask before exp,
and straddle blocks only compute/exp their valid columns.
"""
import sys

sys.path.insert(0, "/opt/trn_rl_repo")

import numpy as np

import concourse.bass as bass
import concourse.tile as tile
from concourse import bacc, mybir
from concourse.bass_utils import run_bass_kernel_spmd

f32 = mybir.dt.float32
f32r = mybir.dt.float32r
EXP = mybir.ActivationFunctionType.Exp

B, S, D, H, HD = 4, 2048, 1024, 16, 64
NCORES = 8
CLOC = D // NCORES       # 128 local channels = 2 heads per core
BS = B * S               # 8192
QT = 4                   # q tiles of 512 per batch
KB = 16                  # k blocks of 128 per batch
NEG = -3.0e38


def build_program():
    """Build + compile the per-core Bacc program (identical on all cores)."""
    nc = bacc.Bacc("TRN2", target_bir_lowering=False, debug=False)

    xtr_d = nc.dram_tensor("xtr", [D, BS], f32r, kind="ExternalInput").ap()
    wq_d = nc.dram_tensor("wq", [D, CLOC], f32, kind="ExternalInput").ap()
    wk_d = nc.dram_tensor("wk", [D, CLOC], f32, kind="ExternalInput").ap()
    wv_d = nc.dram_tensor("wv", [D, CLOC], f32, kind="ExternalInput").ap()
    wo_d = nc.dram_tensor("wo", [CLOC, D], f32, kind="ExternalInput").ap()
    selc_d = nc.dram_tensor("selc", [2, CLOC], f32, kind="ExternalInput").ap()
    out_d = nc.dram_tensor("out", [BS, D], f32, kind="ExternalOutput").ap()

    with tile.TileContext(nc) as tc:
        _Builder(nc, tc, xtr_d, wq_d, wk_d, wv_d, wo_d, selc_d, out_d).build()
    nc.compile()
    return nc


class _Builder:
    def __init__(self, nc, tc, xtr_d, wq_d, wk_d, wv_d, wo_d, selc_d, out_d):
        self.nc = nc
        self.tc = tc
        self.xtr_d = xtr_d
        self.w_d = {"q": wq_d, "k": wk_d, "v": wv_d}
        self.wo_d = wo_d
        self.selc_d = selc_d
        self.out_d = out_d
        self.st_b = {}   # per-batch state: xt, qT, kT, vT, aT, v_tiles
        from collections import deque
        self.fillers = deque()

    def build(self):
        from contextlib import ExitStack

        nc, tc = self.nc, self.tc
        with ExitStack() as ctx:
            p = self.p = {}
            for name, bufs, space in (
                ("consts", 1, None), ("wpool", 1, None), ("xtp", 1, None),
                ("qkv", 2, None), ("vtpool", 1, None), ("vpp", 24, None),
                ("ptp", 4, None), ("atp", 2, None), ("denp", 3, None),
                ("outp", 2, None),
                ("ps_a", 2, "PSUM"), ("ps_st", 2, "PSUM"),
                ("ps_pv", 2, "PSUM"),
            ):
                kw = {"space": space} if space else {}
                p[name] = ctx.enter_context(
                    tc.tile_pool(name=name, bufs=bufs, **kw))

            self._consts()
            self._weights()

            # ---- software pipeline across batches: proj(b+1)/Wo(b)
            # queue as PE "filler" thunks drained inside the attention
            # kb loop so the PE queue never idles on exp waits ----
            self._xt_dma(0)
            for qt in range(QT):
                self._proj_group(0, qt)
            self._vtrans(0)
            for b in range(B):
                if b + 1 < B:
                    self._xt_dma(b + 1)
                    for qt in range(QT):
                        self._enqueue_proj(b + 1, qt)
                for qt in range(QT):
                    self._attention_qtile(b, qt)
                    self._den_part1(b, qt)
                    if qt >= 1:
                        self._den_part2(b, qt - 1)
                        self._enqueue_wo(b, qt - 1)
                self._den_part2(b, QT - 1)
                self._enqueue_wo(b, QT - 1)
                self._drain_fillers()
            self._drain_fillers()

    # ------------------------------------------------------------------
    def _consts(self):
        nc, p = self.nc, self.p
        ident = p["consts"].tile([128, 128], f32)
        nc.gpsimd.memset(ident[:], 0.0)
        nc.gpsimd.affine_select(
            out=ident[:], in_=ident[:],
            compare_op=mybir.AluOpType.not_equal, fill=1.0, base=0,
            pattern=[[-1, 128]], channel_multiplier=1,
        )
        trimask = p["consts"].tile([128, 128], f32)
        nc.gpsimd.memset(trimask[:], 0.0)
        nc.gpsimd.affine_select(
            out=trimask[:], in_=trimask[:],
            compare_op=mybir.AluOpType.is_ge, fill=NEG, base=0,
            pattern=[[1, 128]], channel_multiplier=-1,
        )
        sel_stg = p["consts"].tile([66, 128], f32)
        nc.sync.dma_start(sel_stg[64:66, :], self.selc_d)
        sel = p["consts"].tile([66, 128], f32r)
        nc.vector.tensor_copy(sel[64:66, :], sel_stg[64:66, :])
        ones_c = p["consts"].tile([128, 1], f32)
        nc.vector.memset(ones_c[:], 1.0)
        self.ident, self.trimask, self.sel, self.ones_c = \
            ident, trimask, sel, ones_c

    def _weights(self):
        nc, p = self.nc, self.p
        self.w_sb = {}
        for name in ("q", "k", "v"):
            stg = p["wpool"].tile([128, D], f32, tag="wstg")
            nc.sync.dma_start(
                stg[:].rearrange("p (c m) -> p c m", c=8),
                self.w_d[name].rearrange("(c p) m -> p c m", p=128))
            w_sb = p["wpool"].tile([128, D], f32r, tag="w_" + name)
            nc.vector.tensor_copy(w_sb[:], stg[:])
            self.w_sb[name] = w_sb
        wo_stg = p["wpool"].tile([128, D], f32, tag="wstg")
        nc.sync.dma_start(wo_stg[:], self.wo_d)
        self.wo_sb = p["wpool"].tile([128, D], f32r, tag="w_o")
        nc.vector.tensor_copy(self.wo_sb[:], wo_stg[:])

    def _st(self, b):
        return self.st_b.setdefault(b, {})

    def _xt_dma(self, b):
        nc, p = self.nc, self.p
        xt = p["xtp"].tile([128, 8 * S], f32r, tag="xt")
        self._st(b)["xt"] = xt
        # qt-major sub-DMAs so the first projection group of this batch
        # only waits for its own 8 slices
        for qt in range(QT):
            for dc in range(8):
                nc.sync.dma_start(
                    xt[:, dc * S + qt * 512: dc * S + (qt + 1) * 512],
                    self.xtr_d[dc * 128:(dc + 1) * 128,
                               b * S + qt * 512: b * S + (qt + 1) * 512])

    def _drain_fillers(self, n=None):
        while self.fillers and (n is None or n > 0):
            self.fillers.popleft()()
            if n is not None:
                n -= 1

    def _enqueue_proj(self, b, qt):
        nc, p = self.nc, self.p
        st = self._st(b)
        if "qT" not in st:
            st["qT"] = p["qkv"].tile([128, S], f32r, tag="qT", name="qT")
            st["kT"] = p["qkv"].tile([128, S], f32r, tag="kT", name="kT")
            st["vT"] = p["vtpool"].tile([128, S], f32, tag="vT", name="vT")
        xt = st["xt"]
        for name in ("v", "q", "k"):
            dst = st[{"q": "qT", "k": "kT", "v": "vT"}[name]]
            if name == "v" and "v_tiles" not in st:
                st["v_tiles"] = [None] * KB
            box = {}

            def mk_mm(dc, name=name, box=box, qt=qt, xt=xt):
                def thunk():
                    if dc == 0:
                        box["pps"] = p["ps_a"].tile(
                            [128, 512], f32, tag="ps_a", name="pps")
                    nc.tensor.matmul(
                        box["pps"][:],
                        self.w_sb[name][:, dc * 128:(dc + 1) * 128],
                        xt[:, dc * S + qt * 512: dc * S + (qt + 1) * 512],
                        start=(dc == 0), stop=(dc == 7))
                return thunk

            for dc in range(8):
                self.fillers.append(mk_mm(dc))

            def cp(dst=dst, box=box, qt=qt):
                nc.vector.tensor_copy(
                    dst[:, qt * 512:(qt + 1) * 512], box["pps"][:])

            self.fillers.append(cp)
            if name == "v":
                for kb in range(4 * qt, 4 * qt + 4):
                    self.fillers.append(
                        lambda kb=kb, b=b: self._vtrans_one(b, kb))

    def _enqueue_wo(self, b, qt):
        nc, p = self.nc, self.p
        aT = self._st(b)["aT"]
        for qb in range(4 * qt, 4 * qt + 4):
            def thunk(qb=qb, aT=aT, b=b):
                o_sb = p["outp"].tile([128, 1024], f32, tag="osb",
                                      name="osb")
                for nt in range(2):
                    pout = p["ps_a"].tile([128, 512], f32, tag="ps_a",
                                          name="pout")
                    nc.tensor.matmul(
                        pout[:], aT[:, qb * 128:(qb + 1) * 128],
                        self.wo_sb[:, nt * 512:(nt + 1) * 512],
                        start=True, stop=True)
                    if (qb + nt) % 2 == 0:
                        nc.vector.tensor_copy(
                            o_sb[:, nt * 512:(nt + 1) * 512], pout[:])
                    else:
                        nc.scalar.copy(
                            o_sb[:, nt * 512:(nt + 1) * 512], pout[:])
                nc.sync.dma_start(
                    self.out_d[b * S + qb * 128: b * S + (qb + 1) * 128, :],
                    o_sb[:])
            self.fillers.append(thunk)

    def _proj_group(self, b, qt):
        nc, p = self.nc, self.p
        st = self._st(b)
        if "qT" not in st:
            st["qT"] = p["qkv"].tile([128, S], f32r, tag="qT", name="qT")
            st["kT"] = p["qkv"].tile([128, S], f32r, tag="kT", name="kT")
            st["vT"] = p["vtpool"].tile([128, S], f32, tag="vT", name="vT")
        xt = st["xt"]
        for name, dst in (("q", st["qT"]), ("k", st["kT"]), ("v", st["vT"])):
            pps = p["ps_a"].tile([128, 512], f32, tag="ps_a")
            for dc in range(8):
                nc.tensor.matmul(
                    pps[:], self.w_sb[name][:, dc * 128:(dc + 1) * 128],
                    xt[:, dc * S + qt * 512: dc * S + (qt + 1) * 512],
                    start=(dc == 0), stop=(dc == 7))
            nc.vector.tensor_copy(dst[:, qt * 512:(qt + 1) * 512], pps[:])

    def _vtrans(self, b):
        st = self._st(b)
        st.setdefault("v_tiles", [None] * KB)
        for kb in range(KB):
            self._vtrans_one(b, kb)

    def _vtrans_one(self, b, kb):
        nc, p = self.nc, self.p
        st = self._st(b)
        vT = st["vT"]
        tp2 = p["ps_a"].tile([128, 512], f32, tag="ps_a")
        nc.tensor.transpose(
            tp2[:, 0:128], vT[:, kb * 128:(kb + 1) * 128], self.ident[:])
        vt = p["vpp"].tile([128, 131], f32r, tag="vp")
        # [V_A(0:64) | 1(64) | V_B(65:129) | pad(129, unread) | 1(130)]
        nc.vector.tensor_copy(vt[:, 64:65], self.ones_c[:])
        nc.vector.tensor_copy(vt[:, 130:131], self.ones_c[:])
        nc.vector.tensor_copy(vt[:, 0:64], tp2[:, 0:64])
        nc.vector.tensor_copy(vt[:, 65:129], tp2[:, 64:128])
        st["v_tiles"][kb] = vt

    def _attention_qtile(self, b, qt):
        nc, p = self.nc, self.p
        st = self._st(b)
        qT, kT, v_tiles = st["qT"], st["kT"], st["v_tiles"]
        if "aT" not in st:
            st["aT"] = p["atp"].tile([128, S], f32r, tag="aT", name="aT")
        pvA = p["ps_pv"].tile([128, 512], f32, tag="ps_pv")
        pvB = p["ps_pv"].tile([128, 512], f32, tag="ps_pv")
        st["pv"] = (pvA, pvB)
        nkb = 4 * qt + 4
        for kb in range(nkb):
            off = max(0, (kb - 4 * qt) * 128)
            stp = p["ps_st"].tile([128, 1024], f32, tag="ps_st")
            nc.tensor.matmul(
                stp[:, off:512], kT[0:64, kb * 128:(kb + 1) * 128],
                qT[0:64, qt * 512 + off:(qt + 1) * 512],
                start=True, stop=True)
            nc.tensor.matmul(
                stp[:, 512 + off:1024],
                kT[64:128, kb * 128:(kb + 1) * 128],
                qT[64:128, qt * 512 + off:(qt + 1) * 512],
                start=True, stop=True)
            if kb - 4 * qt >= 0:
                for hoff in (0, 512):
                    nc.vector.tensor_add(
                        stp[:, hoff + off:hoff + off + 128],
                        stp[:, hoff + off:hoff + off + 128],
                        self.trimask[:])
            pt = p["ptp"].tile([128, 1024], f32r, tag="pt")
            st_v = stp[:].rearrange("p (h q) -> p h q", h=2)[:, :, off:512]
            pt_v = pt[:].rearrange("p (h q) -> p h q", h=2)[:, :, off:512]
            nc.scalar.activation(pt_v, st_v, EXP, scale=0.125)
            nc.tensor.matmul(
                pvA[0:65, off:512], v_tiles[kb][:, 0:65], pt[:, off:512],
                start=(kb == 0), stop=(kb == nkb - 1))
            nc.tensor.matmul(
                pvB[0:66, off:512], v_tiles[kb][:, 65:131],
                pt[:, 512 + off:1024],
                start=(kb == 0), stop=(kb == nkb - 1))
            self._drain_fillers(4 if kb % 2 == 0 else 3)

    def _den_part1(self, b, qt):
        nc, p = self.nc, self.p
        st = self._st(b)
        pvA, pvB = st.pop("pv")
        # stage psum out^T -> SBUF and den rows; kick the repartition DMAs
        stgA = p["denp"].tile([128, 512], f32r, tag="stgA")
        nc.scalar.copy(stgA[0:64, :], pvA[0:64, :])
        stgB = p["denp"].tile([128, 512], f32r, tag="stgB")
        nc.scalar.copy(stgB[0:64, :], pvB[0:64, :])
        dens = p["denp"].tile([128, 512], f32, tag="dens")
        nc.vector.tensor_copy(dens[64:66, :], pvB[64:66, :])
        nc.vector.tensor_copy(dens[64:65, :], pvA[64:65, :])
        densP = p["denp"].tile([128, 8], f32, tag="densP")
        for h in range(2):
            for qh in range(4):
                nc.sync.dma_start(
                    densP[:, 4 * h + qh: 4 * h + qh + 1],
                    dens[64 + h: 65 + h, 128 * qh: 128 * (qh + 1)])
        st.setdefault("den_pend", {})[qt] = (stgA, stgB, dens, densP)

    def _den_part2(self, b, qt):
        nc, p = self.nc, self.p
        st = self._st(b)
        aT = st["aT"]
        stgA, stgB, dens, densP = st["den_pend"].pop(qt)
        cols = slice(qt * 512, (qt + 1) * 512)
        nc.vector.reciprocal(densP[:], densP[:])
        for h in range(2):
            for qh in range(4):
                nc.sync.dma_start(
                    dens[64 + h: 65 + h, 128 * qh: 128 * (qh + 1)],
                    densP[:, 4 * h + qh: 4 * h + qh + 1])
        recip_r = p["denp"].tile([128, 512], f32r, tag="recip_r")
        nc.scalar.copy(recip_r[64:66, :], dens[64:66, :])
        r_ps = p["ps_a"].tile([128, 512], f32, tag="ps_a")
        nc.tensor.matmul(r_ps[:], self.sel[64:66, :], recip_r[64:66, :],
                         start=True, stop=True)
        nc.sync.dma_start(aT[64:128, cols], stgB[0:64, :])
        nc.vector.tensor_mul(aT[0:64, cols], stgA[0:64, :], r_ps[0:64, :])
        nc.vector.tensor_mul(aT[64:128, cols], aT[64:128, cols],
                             r_ps[64:128, :])

    def _wo_group(self, b, qt):
        nc, p = self.nc, self.p
        aT = self._st(b)["aT"]
        for qb in range(4 * qt, 4 * qt + 4):
            o_sb = p["outp"].tile([128, 1024], f32, tag="osb")
            for nt in range(2):
                pout = p["ps_a"].tile([128, 512], f32, tag="ps_a")
                nc.tensor.matmul(
                    pout[:], aT[:, qb * 128:(qb + 1) * 128],
                    self.wo_sb[:, nt * 512:(nt + 1) * 512],
                    start=True, stop=True)
                if (qb + nt) % 2 == 0:
                    nc.vector.tensor_copy(
                        o_sb[:, nt * 512:(nt + 1) * 512], pout[:])
                else:
                    nc.scalar.copy(
                        o_sb[:, nt * 512:(nt + 1) * 512], pout[:])
            nc.sync.dma_start(
                self.out_d[b * S + qb * 128: b * S + (qb + 1) * 128, :],
                o_sb[:])


_PROGRAM_CACHE = {}


def _get_program():
    if "nc" not in _PROGRAM_CACHE:
        _PROGRAM_CACHE["nc"] = build_program()
    return _PROGRAM_CACHE["nc"]


def make_in_maps(x, Wq, Wk, Wv, Wo):
    x_flat = np.asarray(x, dtype=np.float32).reshape(BS, D)
    xtr = np.ascontiguousarray(x_flat.T)
    sel_const = np.zeros((2, CLOC), dtype=np.float32)
    sel_const[0, 0:64] = 1.0
    sel_const[1, 64:128] = 1.0
    maps = []
    for c in range(NCORES):
        sl = slice(c * CLOC, (c + 1) * CLOC)
        maps.append({
            "xtr": xtr,
            "wq": np.ascontiguousarray(Wq[:, sl], dtype=np.float32),
            "wk": np.ascontiguousarray(Wk[:, sl], dtype=np.float32),
            "wv": np.ascontiguousarray(Wv[:, sl], dtype=np.float32),
            "wo": np.ascontiguousarray(Wo[sl, :], dtype=np.float32),
            "selc": sel_const,
        })
    return maps


def run(x, Wq, Wk, Wv, Wo, bo, trace=False, **kw):
    nc = _get_program()
    maps = make_in_maps(x, Wq, Wk, Wv, Wo)
    res = run_bass_kernel_spmd(nc, maps, core_ids=list(range(NCORES)),
                               trace=trace, **kw)
    acc = res.results[0]["out"].astype(np.float32)
    for c in range(1, NCORES):
        acc = acc + res.results[c]["out"]
    out = (acc + np.asarray(bo, dtype=np.float32)).reshape(B, S, D)
    return out, res


def kernel(x, Wq, Wk, Wv, Wo, bo):
    out, _ = run(x, Wq, Wk, Wv, Wo, bo, trace=False)
    return out


# revision 33
# speedup vs baseline: 1.2209x; 1.0179x over previous
"""Causal multi-head attention on 8 trn2 NeuronCores.

Sharding: head-parallel. Each core owns 2 of the 16 heads (128 of the 1024
channels) for all 4 batches. Per core:
  Q^T/K^T/V^T projections (local 128 channels) from x^T (host-transposed,
  a pure layout prep like the weight slicing); flash-style causal attention
  in score-transposed layout S^T[k, q]; softmax denominators ride along as a
  ones column appended to V (PV matmul M=65/66, den lands on its own PSUM
  partition); normalization is applied to A^T via a rank-2 "R" matmul built
  from the reciprocals; local Wo row-block matmul produces a full
  [8192, 1024] partial per core, summed (+bias) on host.

Engine-queue discipline (queues are static FIFO on TRN2):
  - The program is software-pipelined at EMISSION level: projections of
    batch b+1 and the Wo of batch b-1 are emitted between the attention
    q-tiles of batch b, so the PE queue interleaves them into the
    ACT-paced attention stretches.
  - The softmax reciprocal (DVE iterative-divide, cost ~ 8 cycles per FREE
    element, partition-parallel) is computed on a [128, 8] repartitioned
    copy of the denominators (tiny DMAs out/back), so it costs ~0.1us of
    DVE queue time instead of 3.3us.

All heavy matmuls run in float32r (tf32-like, full PE rate at free dim >=
256, ~1.5e-4 scale-relative per matmul measured on HW). PE transposes (V^T
-> V) run in plain fp32 (exact). Softmax skips the max-subtraction (scores
are bounded; fp32 exp cannot overflow) and folds the 1/sqrt(64) scale into
the ACT exp. Causal masking is block-wise: off-diagonal key blocks are
skipped entirely; diagonal blocks get a -3e38 triangular mask before exp,
and straddle blocks only compute/exp their valid columns.
"""
import sys

sys.path.insert(0, "/opt/trn_rl_repo")

import numpy as np

import concourse.bass as bass
import concourse.tile as tile
from concourse import bacc, mybir
from concourse.bass_utils import run_bass_kernel_spmd

f32 = mybir.dt.float32
f32r = mybir.dt.float32r
EXP = mybir.ActivationFunctionType.Exp

B, S, D, H, HD = 4, 2048, 1024, 16, 64
NCORES = 8
CLOC = D // NCORES       # 128 local channels = 2 heads per core
BS = B * S               # 8192
QT = 4                   # q tiles of 512 per batch
KB = 16                  # k blocks of 128 per batch
NEG = -3.0e38


def build_program():
    """Build + compile the per-core Bacc program (identical on all cores)."""
    nc = bacc.Bacc("TRN2", target_bir_lowering=False, debug=False)

    xtr_d = nc.dram_tensor("xtr", [D, BS], f32r, kind="ExternalInput").ap()
    wq_d = nc.dram_tensor("wq", [D, CLOC], f32, kind="ExternalInput").ap()
    wk_d = nc.dram_tensor("wk", [D, CLOC], f32, kind="ExternalInput").ap()
    wv_d = nc.dram_tensor("wv", [D, CLOC], f32, kind="ExternalInput").ap()
    wo_d = nc.dram_tensor("wo", [CLOC, D], f32, kind="ExternalInput").ap()
    selc_d = nc.dram_tensor("selc", [2, CLOC], f32, kind="ExternalInput").ap()
    out_d = nc.dram_tensor("out", [BS, D], f32, kind="ExternalOutput").ap()

    with tile.TileContext(nc) as tc:
        _Builder(nc, tc, xtr_d, wq_d, wk_d, wv_d, wo_d, selc_d, out_d).build()
    nc.compile()
    return nc


class _Builder:
    def __init__(self, nc, tc, xtr_d, wq_d, wk_d, wv_d, wo_d, selc_d, out_d):
        self.nc = nc
        self.tc = tc
        self.xtr_d = xtr_d
        self.w_d = {"q": wq_d, "k": wk_d, "v": wv_d}
        self.wo_d = wo_d
        self.selc_d = selc_d
        self.out_d = out_d
        self.st_b = {}   # per-batch state: xt, qT, kT, vT, aT, v_tiles
        from collections import deque
        self.fillers = deque()

    def build(self):
        from contextlib import ExitStack

        nc, tc = self.nc, self.tc
        with ExitStack() as ctx:
            p = self.p = {}
            for name, bufs, space in (
                ("consts", 1, None), ("wpool", 1, None), ("xtp", 1, None),
                ("qkv", 2, None), ("vtpool", 1, None), ("vpp", 24, None),
                ("ptp", 5, None), ("atp", 2, None), ("denp", 3, None),
                ("outp", 2, None),
                ("ps_a", 2, "PSUM"), ("ps_st", 2, "PSUM"),
                ("ps_pv", 2, "PSUM"),
            ):
                kw = {"space": space} if space else {}
                p[name] = ctx.enter_context(
                    tc.tile_pool(name=name, bufs=bufs, **kw))

            self._consts()
            self._weights()

            # ---- software pipeline across batches: proj(b+1)/Wo(b)
            # queue as PE "filler" thunks drained inside the attention
            # kb loop so the PE queue never idles on exp waits ----
            self._xt_dma(0)
            for qt in range(QT):
                self._proj_group(0, qt)
            self._vtrans(0)
            for b in range(B):
                if b + 1 < B:
                    self._xt_dma(b + 1)
                    for qt in range(QT):
                        self._enqueue_proj(b + 1, qt)
                for qt in range(QT):
                    self._attention_qtile(b, qt)
                    self._den_part1(b, qt)
                    if qt >= 1:
                        self._den_part2(b, qt - 1)
                        self._enqueue_wo(b, qt - 1)
                self._den_part2(b, QT - 1)
                self._enqueue_wo(b, QT - 1)
                self._drain_fillers()
            self._drain_fillers()

    # ------------------------------------------------------------------
    def _consts(self):
        nc, p = self.nc, self.p
        ident = p["consts"].tile([128, 128], f32)
        nc.gpsimd.memset(ident[:], 0.0)
        nc.gpsimd.affine_select(
            out=ident[:], in_=ident[:],
            compare_op=mybir.AluOpType.not_equal, fill=1.0, base=0,
            pattern=[[-1, 128]], channel_multiplier=1,
        )
        trimask = p["consts"].tile([128, 128], f32)
        nc.gpsimd.memset(trimask[:], 0.0)
        nc.gpsimd.affine_select(
            out=trimask[:], in_=trimask[:],
            compare_op=mybir.AluOpType.is_ge, fill=NEG, base=0,
            pattern=[[1, 128]], channel_multiplier=-1,
        )
        sel_stg = p["consts"].tile([66, 128], f32)
        nc.sync.dma_start(sel_stg[64:66, :], self.selc_d)
        sel = p["consts"].tile([66, 128], f32r)
        nc.vector.tensor_copy(sel[64:66, :], sel_stg[64:66, :])
        ones_c = p["consts"].tile([128, 1], f32)
        nc.vector.memset(ones_c[:], 1.0)
        self.ident, self.trimask, self.sel, self.ones_c = \
            ident, trimask, sel, ones_c

    def _weights(self):
        nc, p = self.nc, self.p
        self.w_sb = {}
        for name in ("q", "k", "v"):
            stg = p["wpool"].tile([128, D], f32, tag="wstg")
            nc.sync.dma_start(
                stg[:].rearrange("p (c m) -> p c m", c=8),
                self.w_d[name].rearrange("(c p) m -> p c m", p=128))
            w_sb = p["wpool"].tile([128, D], f32r, tag="w_" + name)
            nc.vector.tensor_copy(w_sb[:], stg[:])
            self.w_sb[name] = w_sb
        wo_stg = p["wpool"].tile([128, D], f32, tag="wstg")
        nc.sync.dma_start(wo_stg[:], self.wo_d)
        self.wo_sb = p["wpool"].tile([128, D], f32r, tag="w_o")
        nc.vector.tensor_copy(self.wo_sb[:], wo_stg[:])

    def _st(self, b):
        return self.st_b.setdefault(b, {})

    def _xt_dma(self, b):
        nc, p = self.nc, self.p
        xt = p["xtp"].tile([128, 8 * S], f32r, tag="xt")
        self._st(b)["xt"] = xt
        # qt-major sub-DMAs so the first projection group of this batch
        # only waits for its own 8 slices
        for qt in range(QT):
            for dc in range(8):
                nc.sync.dma_start(
                    xt[:, dc * S + qt * 512: dc * S + (qt + 1) * 512],
                    self.xtr_d[dc * 128:(dc + 1) * 128,
                               b * S + qt * 512: b * S + (qt + 1) * 512])

    def _drain_fillers(self, n=None):
        while self.fillers and (n is None or n > 0):
            self.fillers.popleft()()
            if n is not None:
                n -= 1

    def _enqueue_proj(self, b, qt):
        nc, p = self.nc, self.p
        st = self._st(b)
        if "qT" not in st:
            st["qT"] = p["qkv"].tile([128, S], f32r, tag="qT", name="qT")
            st["kT"] = p["qkv"].tile([128, S], f32r, tag="kT", name="kT")
            st["vT"] = p["vtpool"].tile([128, S], f32, tag="vT", name="vT")
        xt = st["xt"]
        for name in ("v", "q", "k"):
            dst = st[{"q": "qT", "k": "kT", "v": "vT"}[name]]
            if name == "v" and "v_tiles" not in st:
                st["v_tiles"] = [None] * KB
            box = {}

            def mk_mm(dc, name=name, box=box, qt=qt, xt=xt):
                def thunk():
                    if dc == 0:
                        box["pps"] = p["ps_a"].tile(
                            [128, 512], f32, tag="ps_a", name="pps")
                    nc.tensor.matmul(
                        box["pps"][:],
                        self.w_sb[name][:, dc * 128:(dc + 1) * 128],
                        xt[:, dc * S + qt * 512: dc * S + (qt + 1) * 512],
                        start=(dc == 0), stop=(dc == 7))
                return thunk

            for dc in range(8):
                self.fillers.append(mk_mm(dc))

            def cp(dst=dst, box=box, qt=qt):
                nc.vector.tensor_copy(
                    dst[:, qt * 512:(qt + 1) * 512], box["pps"][:])

            self.fillers.append(cp)
            if name == "v":
                for kb in range(4 * qt, 4 * qt + 4):
                    self.fillers.append(
                        lambda kb=kb, b=b: self._vtrans_one(b, kb))

    def _enqueue_wo(self, b, qt):
        nc, p = self.nc, self.p
        aT = self._st(b)["aT"]
        for qb in range(4 * qt, 4 * qt + 4):
            def thunk(qb=qb, aT=aT, b=b):
                o_sb = p["outp"].tile([128, 1024], f32, tag="osb",
                                      name="osb")
                for nt in range(2):
                    pout = p["ps_a"].tile([128, 512], f32, tag="ps_a",
                                          name="pout")
                    nc.tensor.matmul(
                        pout[:], aT[:, qb * 128:(qb + 1) * 128],
                        self.wo_sb[:, nt * 512:(nt + 1) * 512],
                        start=True, stop=True)
                    if (qb + nt) % 2 == 0:
                        nc.vector.tensor_copy(
                            o_sb[:, nt * 512:(nt + 1) * 512], pout[:])
                    else:
                        nc.scalar.copy(
                            o_sb[:, nt * 512:(nt + 1) * 512], pout[:])
                nc.sync.dma_start(
                    self.out_d[b * S + qb * 128: b * S + (qb + 1) * 128, :],
                    o_sb[:])
            self.fillers.append(thunk)

    def _proj_group(self, b, qt):
        nc, p = self.nc, self.p
        st = self._st(b)
        if "qT" not in st:
            st["qT"] = p["qkv"].tile([128, S], f32r, tag="qT", name="qT")
            st["kT"] = p["qkv"].tile([128, S], f32r, tag="kT", name="kT")
            st["vT"] = p["vtpool"].tile([128, S], f32, tag="vT", name="vT")
        xt = st["xt"]
        for name, dst in (("q", st["qT"]), ("k", st["kT"]), ("v", st["vT"])):
            pps = p["ps_a"].tile([128, 512], f32, tag="ps_a")
            for dc in range(8):
                nc.tensor.matmul(
                    pps[:], self.w_sb[name][:, dc * 128:(dc + 1) * 128],
                    xt[:, dc * S + qt * 512: dc * S + (qt + 1) * 512],
                    start=(dc == 0), stop=(dc == 7))
            nc.vector.tensor_copy(dst[:, qt * 512:(qt + 1) * 512], pps[:])

    def _vtrans(self, b):
        st = self._st(b)
        st.setdefault("v_tiles", [None] * KB)
        for kb in range(KB):
            self._vtrans_one(b, kb)

    def _vtrans_one(self, b, kb):
        nc, p = self.nc, self.p
        st = self._st(b)
        vT = st["vT"]
        tp2 = p["ps_a"].tile([128, 512], f32, tag="ps_a")
        nc.tensor.transpose(
            tp2[:, 0:128], vT[:, kb * 128:(kb + 1) * 128], self.ident[:])
        vt = p["vpp"].tile([128, 131], f32r, tag="vp")
        # [V_A(0:64) | 1(64) | V_B(65:129) | pad(129, unread) | 1(130)]
        nc.vector.tensor_copy(vt[:, 64:65], self.ones_c[:])
        nc.vector.tensor_copy(vt[:, 130:131], self.ones_c[:])
        nc.vector.tensor_copy(vt[:, 0:64], tp2[:, 0:64])
        nc.vector.tensor_copy(vt[:, 65:129], tp2[:, 64:128])
        st["v_tiles"][kb] = vt

    def _attention_qtile(self, b, qt):
        nc, p = self.nc, self.p
        st = self._st(b)
        qT, kT, v_tiles = st["qT"], st["kT"], st["v_tiles"]
        if "aT" not in st:
            st["aT"] = p["atp"].tile([128, S], f32r, tag="aT", name="aT")
        pvA = p["ps_pv"].tile([128, 512], f32, tag="ps_pv")
        pvB = p["ps_pv"].tile([128, 512], f32, tag="ps_pv")
        st["pv"] = (pvA, pvB)
        nkb = 4 * qt + 4
        for kb in range(nkb):
            off = max(0, (kb - 4 * qt) * 128)
            stp = p["ps_st"].tile([128, 1024], f32, tag="ps_st")
            nc.tensor.matmul(
                stp[:, off:512], kT[0:64, kb * 128:(kb + 1) * 128],
                qT[0:64, qt * 512 + off:(qt + 1) * 512],
                start=True, stop=True)
            nc.tensor.matmul(
                stp[:, 512 + off:1024],
                kT[64:128, kb * 128:(kb + 1) * 128],
                qT[64:128, qt * 512 + off:(qt + 1) * 512],
                start=True, stop=True)
            if kb - 4 * qt >= 0:
                for hoff in (0, 512):
                    nc.vector.tensor_add(
                        stp[:, hoff + off:hoff + off + 128],
                        stp[:, hoff + off:hoff + off + 128],
                        self.trimask[:])
            pt = p["ptp"].tile([128, 1024], f32r, tag="pt")
            st_v = stp[:].rearrange("p (h q) -> p h q", h=2)[:, :, off:512]
            pt_v = pt[:].rearrange("p (h q) -> p h q", h=2)[:, :, off:512]
            nc.scalar.activation(pt_v, st_v, EXP, scale=0.125)
            nc.tensor.matmul(
                pvA[0:65, off:512], v_tiles[kb][:, 0:65], pt[:, off:512],
                start=(kb == 0), stop=(kb == nkb - 1))
            nc.tensor.matmul(
                pvB[0:66, off:512], v_tiles[kb][:, 65:131],
                pt[:, 512 + off:1024],
                start=(kb == 0), stop=(kb == nkb - 1))
            self._drain_fillers(4 if kb % 2 == 0 else 3)

    def _den_part1(self, b, qt):
        nc, p = self.nc, self.p
        st = self._st(b)
        pvA, pvB = st.pop("pv")
        # stage psum out^T -> SBUF and den rows; kick the repartition DMAs
        stgA = p["denp"].tile([128, 512], f32r, tag="stgA")
        nc.scalar.copy(stgA[0:64, :], pvA[0:64, :])
        stgB = p["denp"].tile([128, 512], f32r, tag="stgB")
        nc.scalar.copy(stgB[0:64, :], pvB[0:64, :])
        dens = p["denp"].tile([128, 512], f32, tag="dens")
        nc.vector.tensor_copy(dens[64:66, :], pvB[64:66, :])
        nc.vector.tensor_copy(dens[64:65, :], pvA[64:65, :])
        densP = p["denp"].tile([128, 8], f32, tag="densP")
        for h in range(2):
            for qh in range(4):
                nc.sync.dma_start(
                    densP[:, 4 * h + qh: 4 * h + qh + 1],
                    dens[64 + h: 65 + h, 128 * qh: 128 * (qh + 1)])
        st.setdefault("den_pend", {})[qt] = (stgA, stgB, dens, densP)

    def _den_part2(self, b, qt):
        nc, p = self.nc, self.p
        st = self._st(b)
        aT = st["aT"]
        stgA, stgB, dens, densP = st["den_pend"].pop(qt)
        cols = slice(qt * 512, (qt + 1) * 512)
        nc.vector.reciprocal(densP[:], densP[:])
        for h in range(2):
            for qh in range(4):
                nc.sync.dma_start(
                    dens[64 + h: 65 + h, 128 * qh: 128 * (qh + 1)],
                    densP[:, 4 * h + qh: 4 * h + qh + 1])
        recip_r = p["denp"].tile([128, 512], f32r, tag="recip_r")
        nc.scalar.copy(recip_r[64:66, :], dens[64:66, :])
        r_ps = p["ps_a"].tile([128, 512], f32, tag="ps_a")
        nc.tensor.matmul(r_ps[:], self.sel[64:66, :], recip_r[64:66, :],
                         start=True, stop=True)
        nc.sync.dma_start(aT[64:128, cols], stgB[0:64, :])
        nc.vector.tensor_mul(aT[0:64, cols], stgA[0:64, :], r_ps[0:64, :])
        nc.vector.tensor_mul(aT[64:128, cols], aT[64:128, cols],
                             r_ps[64:128, :])

    def _wo_group(self, b, qt):
        nc, p = self.nc, self.p
        aT = self._st(b)["aT"]
        for qb in range(4 * qt, 4 * qt + 4):
            o_sb = p["outp"].tile([128, 1024], f32, tag="osb")
            for nt in range(2):
                pout = p["ps_a"].tile([128, 512], f32, tag="ps_a")
                nc.tensor.matmul(
                    pout[:], aT[:, qb * 128:(qb + 1) * 128],
                    self.wo_sb[:, nt * 512:(nt + 1) * 512],
                    start=True, stop=True)
                if (qb + nt) % 2 == 0:
                    nc.vector.tensor_copy(
                        o_sb[:, nt * 512:(nt + 1) * 512], pout[:])
                else:
                    nc.scalar.copy(
                        o_sb[:, nt * 512:(nt + 1) * 512], pout[:])
            nc.sync.dma_start(
                self.out_d[b * S + qb * 128: b * S + (qb + 1) * 128, :],
                o_sb[:])


_PROGRAM_CACHE = {}


def _get_program():
    if "nc" not in _PROGRAM_CACHE:
        _PROGRAM_CACHE["nc"] = build_program()
    return _PROGRAM_CACHE["nc"]


def make_in_maps(x, Wq, Wk, Wv, Wo):
    x_flat = np.asarray(x, dtype=np.float32).reshape(BS, D)
    xtr = np.ascontiguousarray(x_flat.T)
    sel_const = np.zeros((2, CLOC), dtype=np.float32)
    sel_const[0, 0:64] = 1.0
    sel_const[1, 64:128] = 1.0
    maps = []
    for c in range(NCORES):
        sl = slice(c * CLOC, (c + 1) * CLOC)
        maps.append({
            "xtr": xtr,
            "wq": np.ascontiguousarray(Wq[:, sl], dtype=np.float32),
            "wk": np.ascontiguousarray(Wk[:, sl], dtype=np.float32),
            "wv": np.ascontiguousarray(Wv[:, sl], dtype=np.float32),
            "wo": np.ascontiguousarray(Wo[sl, :], dtype=np.float32),
            "selc": sel_const,
        })
    return maps


def run(x, Wq, Wk, Wv, Wo, bo, trace=False, **kw):
    nc = _get_program()
    maps = make_in_maps(x, Wq, Wk, Wv, Wo)
    res = run_bass_kernel_spmd(nc, maps, core_ids=list(range(NCORES)),
                               trace=trace, **kw)
    acc = res.results[0]["out"].astype(np.float32)
    for c in range(1, NCORES):
        acc = acc + res.results[c]["out"]
    out = (acc + np.asarray(bo, dtype=np.float32)).reshape(B, S, D)
    return out, res


def kernel(x, Wq, Wk, Wv, Wo, bo):
    out, _ = run(x, Wq, Wk, Wv, Wo, bo, trace=False)
    return out
